# revision 9
# baseline (speedup 1.0000x reference)
"""GQA attention (B=2, T=2048, D=4096, H=32, G=8, d=128) on 8 TRN2 NeuronCores.

Sharding: one KV group per core (4 Q heads + 1 K/V head). Each core:
  - projects q/k/v for its group in transposed ("d-major") layout,
  - causal attention with transposed score tiles S.T = K.T-chunk @ Q-tile.
    Non-diagonal k-tiles are processed in pairs (2 k-tiles per 2-bank psum
    tile, one exp per pair); diagonal k-tiles are singles narrowed to the
    valid trapezoid, with the causal triangle applied as a post-exp
    multiply by a 0/1 triangle tile on DVE (no PE mask matmuls).
    Softmax row-sums accumulate in bf16 on DVE; one ones-matmul pair per
    chain turns them into denominators.
  - out-projection groups are interleaved into the attention phase as
    token tiles complete (loop order b,qi,h makes tiles ready early), so
    the PE stays dense through exp/DVE dependency bubbles; leftovers
    drain in a final phase.
Host sums the 8 partial outputs in f32 and adds bo.

All matmuls in bf16 with fp32 PSUM accumulation.
"""

import math
import sys

import numpy as np

sys.path.insert(0, "/opt/trn_rl_repo")

import ml_dtypes

BF16 = ml_dtypes.bfloat16

B, T, D = 2, 2048, 4096
H, G, d = 32, 8, 128
GROUP = H // G  # 4 heads per group/core
NT = B * T  # 4096 tokens
NC_ = 8  # cores

TOK = 512  # q-token tile (free dim of score matmuls, psum bank)
NTT = NT // TOK  # 8
DC = D // 128  # 32 contraction chunks
XC = 4  # Dc chunks per x DMA chunk (512KB each)
NXC = DC // XC  # 8 x-chunks per token tile
KT_PER_B = T // 128  # 16 k-tiles per batch

_program_cache = {}


def _build_program():
    import concourse.mybir as mybir
    import concourse.tile as tile
    from concourse import bacc
    from concourse.bass import ds, ts
    from concourse.masks import make_identity

    f32 = mybir.dt.float32
    bf16 = mybir.dt.bfloat16
    AF = mybir.ActivationFunctionType

    nc = bacc.Bacc()

    xt_d = nc.declare_dram_parameter("xt", [128, NTT, DC, TOK], bf16, isOutput=False)
    wq_d = nc.declare_dram_parameter("wq", [128, DC * GROUP, 128], bf16, isOutput=False)
    wk_d = nc.declare_dram_parameter("wk", [128, DC, 128], bf16, isOutput=False)
    wv_d = nc.declare_dram_parameter("wv", [128, DC, 128], bf16, isOutput=False)
    wo_d = nc.declare_dram_parameter("wo", [128, GROUP * DC, 128], bf16, isOutput=False)
    bq_d = nc.declare_dram_parameter("bq", [128, GROUP], f32, isOutput=False)
    bk_d = nc.declare_dram_parameter("bk", [128, 1], f32, isOutput=False)
    bv_d = nc.declare_dram_parameter("bv", [128, 1], f32, isOutput=False)
    tri_d = nc.declare_dram_parameter("tri", [128, 128], bf16, isOutput=False)
    out_d = nc.declare_dram_parameter("out", [128, DC, NT], bf16, isOutput=True)

    with tile.TileContext(nc) as tc:
        with tc.tile_pool(name="persist", bufs=1) as persist:
            qT = persist.tile([128, GROUP, NT], bf16)  # [dq_row, head, tok]
            kT = persist.tile([128, NT], bf16)  # [d, tok]
            vtm = persist.tile([128, NT // 128, 128], bf16)  # [tok%128, tile, dv]
            yT = persist.tile([128, GROUP, NT], bf16)  # [dv, head, tok]
            tri01 = persist.tile([128, 128], bf16)
            bq_s = persist.tile([128, GROUP], f32)
            bk_s = persist.tile([128, 1], f32)
            bv_s = persist.tile([128, 1], f32)
            ones128 = persist.tile([128, 128], bf16)
            ident = persist.tile([128, 128], bf16)

            # ---------------- Phase 1: q/k/v projections ----------------
            with (
                tc.tile_pool(name="wproj", bufs=1) as wpool,
                tc.tile_pool(name="xtp", bufs=16) as xpool,
                tc.tile_pool(name="vtstage", bufs=1) as vtp,
                tc.psum_pool(name="ps1", bufs=6) as ps1,
                tc.psum_pool(name="pstr", bufs=2) as pstr,
            ):
                wk_s = wpool.tile([128, DC, 128], bf16)
                wv_s = wpool.tile([128, DC, 128], bf16)
                wq_s = wpool.tile([128, GROUP * DC, 128], bf16)

                # critical-path-first DMA ordering: the very first matmuls
                # need only wk chunk 0 + x chunk (0,0); v-proj of tile 0
                # needs wv shortly after; q-proj of tile 0 needs wq head 0
                # before the second x tile.
                xch = {}

                def xdma(tt, c):
                    xc = xpool.tile([128, XC, TOK], bf16, tag="xt")
                    xch[(tt, c)] = xc
                    nc.sync.dma_start(
                        out=xc[:], in_=xt_d[:, tt, c * XC : (c + 1) * XC, :]
                    )

                # first chunk arrives Dc-by-Dc so the very first matmul
                # only waits on 32KB + 128KB
                xc00 = xpool.tile([128, XC, TOK], bf16, tag="xt")
                xch[(0, 0)] = xc00
                for i in range(XC):
                    nc.sync.dma_start(
                        out=wk_s[:, i : i + 1, :], in_=wk_d[:, i : i + 1, :]
                    )
                    nc.sync.dma_start(
                        out=xc00[:, i, :], in_=xt_d[:, 0, i, :]
                    )
                # wv chunks ride along with the x(0) stream so v-proj of
                # tile 0 never waits; all wq heads go before x(1), which
                # isn't needed until ~40us in.
                for cc in range(1, NXC):
                    nc.sync.dma_start(
                        out=wk_s[:, cc * XC : (cc + 1) * XC, :],
                        in_=wk_d[:, cc * XC : (cc + 1) * XC, :],
                    )
                    xdma(0, cc)
                    if cc % 2 == 1:
                        w0 = (cc // 2) * XC * 2
                        nc.sync.dma_start(
                            out=wv_s[:, w0 : w0 + 2 * XC, :],
                            in_=wv_d[:, w0 : w0 + 2 * XC, :],
                        )
                nc.sync.dma_start(out=bk_s[:], in_=bk_d[:])
                nc.sync.dma_start(out=bv_s[:], in_=bv_d[:])
                nc.sync.dma_start(out=bq_s[:], in_=bq_d[:])
                for dq in range(GROUP):
                    nc.sync.dma_start(
                        out=wq_s[:, ts(dq, DC), :], in_=wq_d[:, ts(dq, DC), :]
                    )
                for c in range(NXC):
                    xdma(1, c)
                nc.sync.dma_start(out=tri01[:], in_=tri_d[:])
                nc.vector.memset(ones128[:], 1.0)
                make_identity(nc, ident[:])
                vT = vtp.tile([128, NT], bf16)

                # PE pre-warm: ~30 dummy matmuls (~6.4us cold) inside the
                # dead DMA-startup window so the HAM clock-gate opens
                # (1.2->2.4 GHz) before the first real matmul at ~10us.
                warm_ps = ps1.tile([128, TOK], f32, tag="ps")
                for _ in range(30):
                    nc.tensor.matmul(
                        warm_ps[:, 0:128],
                        lhsT=ones128[:],
                        rhs=ones128[:],
                        start=True,
                        stop=True,
                        skip_group_check=True,
                    )

                def xsrc(tt, Dc):
                    return xch[(tt, Dc // XC)][:, Dc % XC, :]

                def proj(tt, w_s, dq, out, bias):
                    ps = ps1.tile([128, TOK], f32, tag="ps")
                    for Dc in range(DC):
                        nc.tensor.matmul(
                            ps[:],
                            lhsT=w_s[:, dq * DC + Dc, :],
                            rhs=xsrc(tt, Dc),
                            start=(Dc == 0),
                            stop=(Dc == DC - 1),
                        )
                    nc.scalar.activation(
                        out=out, in_=ps[:], func=AF.Identity, bias=bias
                    )

                sched = []
                for tt in range(NTT):
                    sched.append(("kv", tt))
                    sched.append(("q", tt))

                for kind, tt in sched:
                    if kind == "kv":
                        if tt + 2 < NTT:  # prefetch x two tiles ahead
                            for c in range(NXC):
                                xdma(tt + 2, c)
                        proj(tt, wk_s, 0, kT[:, ts(tt, TOK)], bk_s[:, 0:1])
                        proj(tt, wv_s, 0, vT[:, ts(tt, TOK)], bv_s[:, 0:1])
                    else:
                        for dq in range(GROUP):
                            proj(
                                tt,
                                wq_s,
                                dq,
                                qT[:, dq, ts(tt, TOK)],
                                bq_s[:, dq : dq + 1],
                            )

                # transpose v to token-major tiles
                for t in range(NT // 128):
                    pt = pstr.tile([128, 128], bf16)
                    nc.tensor.transpose(pt[:], vT[:, ts(t, 128)], ident[:])
                    nc.vector.tensor_copy(out=vtm[:, t, :], in_=pt[:])

            # -------- Phase 2: causal attention + interleaved out-proj --------
            with (
                tc.tile_pool(name="wout", bufs=1) as wop,
                tc.tile_pool(name="stg", bufs=10) as stg,
            ):
                wo_s = wop.tile([128, GROUP * DC, 128], bf16)
                nc.sync.dma_start(out=wo_s[:], in_=wo_d[:])

                oproj_done = set()
                drain_ctr = [0]

                def emit_oproj_group(pool, tt, Do):
                    # one out-projection psum group + stage-copy + DMA out
                    oproj_done.add((tt, Do))
                    ps = pool.tile([128, TOK], f32, tag="lp")
                    for c in range(GROUP):
                        nc.tensor.matmul(
                            ps[:],
                            lhsT=wo_s[:, c * DC + Do, :],
                            rhs=yT[:, c, ts(tt, TOK)],
                            start=(c == 0),
                            stop=(c == GROUP - 1),
                        )
                    so = stg.tile([128, TOK], bf16, tag="so")
                    if drain_ctr[0] % 2 == 0:
                        nc.vector.tensor_copy(out=so[:], in_=ps[:])
                    else:
                        nc.scalar.copy(out=so[:], in_=ps[:])
                    drain_ctr[0] += 1
                    nc.sync.dma_start(out=out_d[:, Do, ts(tt, TOK)], in_=so[:])

                with (
                    tc.psum_pool(name="att_st", bufs=2) as stp,
                    tc.psum_pool(name="att_y", bufs=2) as yp,
                    tc.psum_pool(name="att_l", bufs=2) as lop,
                    tc.tile_pool(name="ptile", bufs=6) as ppool,
                    tc.tile_pool(name="acc", bufs=3) as accp,
                    tc.tile_pool(name="invl", bufs=3) as invp,
                ):
                    # out-projection groups become ready per token tile as
                    # its 4 chains' tails fire; feed them into the PE stream
                    # to fill the exp/DVE-latency bubbles.
                    oproj_ready = []
                    oproj_cursor = [0]

                    def emit_early_oproj(n):
                        while n > 0 and oproj_cursor[0] < len(oproj_ready):
                            tt, Do = oproj_ready[oproj_cursor[0]]
                            oproj_cursor[0] += 1
                            emit_oproj_group(lop, tt, Do)
                            n -= 1

                    # pending chain tail: (emit_fn, tile_to_mark_ready|None)
                    pending_tail = [None]

                    def fire_tail():
                        if pending_tail[0] is not None:
                            fn, done_tt = pending_tail[0]
                            fn()
                            pending_tail[0] = None
                            if done_tt is not None:
                                for Do in range(DC):
                                    oproj_ready.append((done_tt, Do))

                    def chain_tail(b, h, qi, yps, accF):
                        # row sums via a single ones-matmul on the folded
                        # accumulator, then 1/l scale. Deferred past the
                        # next chain's start so the PE isn't stalled on the
                        # DVE accumulate latency.
                        def tail():
                            lps = lop.tile([128, TOK], f32, tag="lp")
                            nc.tensor.matmul(
                                lps[:],
                                lhsT=ones128[:],
                                rhs=accF[:],
                                start=True,
                                stop=True,
                            )
                            inv = invp.tile([128, TOK], f32)
                            nc.vector.reciprocal_approx_fast(
                                out=inv[:], in_=lps[:]
                            )
                            nc.vector.tensor_mul(
                                out=yT[:, h, ds(b * T + qi * TOK, TOK)],
                                in0=yps[:],
                                in1=inv[:],
                            )

                        return tail

                    chains = [
                        (b, qi, h)
                        for b in range(B)
                        for qi in range(T // TOK)
                        for h in range(GROUP)
                    ]

                    for b, qi, h in chains:
                        njt = 4 * (qi + 1)  # causal k-tiles of 128
                        yps = yp.tile([128, TOK], f32)
                        accEO = accp.tile([128, 2, TOK], bf16)
                        acc_init = [False, False]
                        if qi == 0:
                            # row 1 gets no k-tile 0 contribution; zero the
                            # first 128 cols so the fold add reads zeros.
                            nc.vector.memset(accEO[:, 1, 0:128], 0.0)

                        # pending AV units: (j, ptile_ap, nlo)
                        pending_av = []

                        def emit_av(j, pap, nlo):
                            nc.tensor.matmul(
                                yps[:, nlo:TOK],
                                lhsT=vtm[:, b * KT_PER_B + j, :],
                                rhs=pap[:, nlo:TOK],
                                start=(j == 0),
                                stop=(j == njt - 1),
                                skip_group_check=(nlo > 0 or j == njt - 1),
                            )

                        def push_av(unit):
                            pending_av.append(unit)
                            if len(pending_av) > 2:
                                emit_av(*pending_av.pop(0))
                                emit_early_oproj(1)

                        unit_idx = 0
                        # non-diagonal k-tile pairs
                        for jp in range(2 * qi):
                            st = stp.tile([128, 2, TOK], f32, tag="st")
                            for jj in range(2):
                                j = jp * 2 + jj
                                nc.tensor.matmul(
                                    st[:, jj, :],
                                    lhsT=kT[:, ds(b * T + j * 128, 128)],
                                    rhs=qT[:, h, ds(b * T + qi * TOK, TOK)],
                                    start=True,
                                    stop=True,
                                )
                            ptile = ppool.tile([128, 2, TOK], bf16, tag="pt")
                            nc.scalar.activation(
                                out=ptile[:], in_=st[:], func=AF.Exp
                            )
                            if not acc_init[0]:
                                nc.vector.tensor_copy(out=accEO[:], in_=ptile[:])
                                acc_init = [True, True]
                            else:
                                nc.vector.tensor_add(
                                    out=accEO[:], in0=ptile[:], in1=accEO[:]
                                )
                            push_av((jp * 2, ptile[:, 0, :], 0))
                            push_av((jp * 2 + 1, ptile[:, 1, :], 0))
                            if jp == 0:
                                fire_tail()
                                emit_early_oproj(2)
                            unit_idx += 1

                        # diagonal k-tiles as narrowed singles
                        for r in range(4):
                            j = 4 * qi + r
                            nlo = r * 128
                            st = stp.tile([128, 2, TOK], f32, tag="st")
                            nc.tensor.matmul(
                                st[:, 0, nlo:TOK],
                                lhsT=kT[:, ds(b * T + j * 128, 128)],
                                rhs=qT[:, h, ds(b * T + qi * TOK + nlo, TOK - nlo)],
                                start=True,
                                stop=True,
                                skip_group_check=True,
                            )
                            ptile = ppool.tile([128, 2, TOK], bf16, tag="pt")
                            nc.scalar.activation(
                                out=ptile[:, 0, nlo:TOK],
                                in_=st[:, 0, nlo:TOK],
                                func=AF.Exp,
                            )
                            # causal triangle: zero future keys in the
                            # 128x128 diagonal block (post-exp 0/1 mask)
                            nc.vector.tensor_mul(
                                out=ptile[:, 0, nlo : nlo + 128],
                                in0=ptile[:, 0, nlo : nlo + 128],
                                in1=tri01[:],
                            )
                            e = r % 2
                            if acc_init[e]:
                                nc.vector.tensor_add(
                                    out=accEO[:, e, nlo:TOK],
                                    in0=ptile[:, 0, nlo:TOK],
                                    in1=accEO[:, e, nlo:TOK],
                                )
                            else:
                                nc.vector.tensor_copy(
                                    out=accEO[:, e, nlo:TOK],
                                    in_=ptile[:, 0, nlo:TOK],
                                )
                                acc_init[e] = True
                            push_av((j, ptile[:, 0, :], nlo))
                            if qi == 0 and r == 0:
                                fire_tail()
                                emit_early_oproj(2)
                            unit_idx += 1

                        for unit in pending_av:
                            emit_av(*unit)
                        # fold the two accumulator rows now (DVE, inputs
                        # ready) so the deferred ones-matmul never waits
                        accF = accp.tile([128, TOK], bf16, tag="accF")
                        nc.vector.tensor_add(
                            out=accF[:], in0=accEO[:, 0, :], in1=accEO[:, 1, :]
                        )
                        done_tt = b * 4 + qi if h == GROUP - 1 else None
                        pending_tail[0] = (chain_tail(b, h, qi, yps, accF), done_tt)
                    fire_tail()

                # ---------------- Phase 3: remaining out-projection ----------------
                with tc.psum_pool(name="ps3", bufs=6) as ps3:
                    for tt in range(NTT):
                        for Do in range(DC):
                            if (tt, Do) not in oproj_done:
                                emit_oproj_group(ps3, tt, Do)

    if not nc.is_finalized():
        nc.finalize()
    return nc


def _prep_inputs(hidden_states, Wq, bq, Wk, bk, Wv, bv, Wo, bo):
    scale = 1.0 / math.sqrt(d)

    x_flat = np.asarray(hidden_states, dtype=np.float32).reshape(NT, D)
    # xt[p, tt, Dc, t'] = x[tt*TOK+t', Dc*128+p]  (token-tile contiguous so
    # each x DMA moves 4KB-per-partition contiguous lines)
    xt = np.ascontiguousarray(
        x_flat.reshape(NTT, TOK, DC, 128).transpose(3, 0, 2, 1)
    ).astype(BF16)

    # tri01[p, c] = 1.0 if key-offset p <= query-offset c else 0.0
    pp = np.arange(128)[:, None]
    cc = np.arange(128)[None, :]
    tri01 = (pp <= cc).astype(np.float32).astype(BF16)
    tri01 = np.ascontiguousarray(tri01)

    in_maps = []
    for g in range(NC_):
        Wq_g = np.asarray(Wq[g * 512 : (g + 1) * 512, :], dtype=np.float32) * scale
        bq_g = np.asarray(bq[g * 512 : (g + 1) * 512], dtype=np.float32) * scale
        Wk_g = np.asarray(Wk[g * 128 : (g + 1) * 128, :], dtype=np.float32)
        bk_g = np.asarray(bk[g * 128 : (g + 1) * 128], dtype=np.float32)
        Wv_g = np.asarray(Wv[g * 128 : (g + 1) * 128], dtype=np.float32)
        bv_g = np.asarray(bv[g * 128 : (g + 1) * 128], dtype=np.float32)
        Wo_g = np.asarray(Wo[:, g * 512 : (g + 1) * 512], dtype=np.float32)

        # wq[p, dq*DC+Dc, m] = Wq_g[dq*128+m, Dc*128+p]
        wq_host = np.ascontiguousarray(
            Wq_g.reshape(GROUP, 128, DC, 128).transpose(3, 0, 2, 1).reshape(
                128, GROUP * DC, 128
            )
        ).astype(BF16)
        # wk[p, Dc, m] = Wk_g[m, Dc*128+p]
        wk_host = np.ascontiguousarray(
            Wk_g.reshape(128, DC, 128).transpose(2, 1, 0)
        ).astype(BF16)
        wv_host = np.ascontiguousarray(
            Wv_g.reshape(128, DC, 128).transpose(2, 1, 0)
        ).astype(BF16)
        # wo[p, c*DC+Do, m] = Wo_g[Do*128+m, c*128+p]
        wo_host = np.ascontiguousarray(
            Wo_g.reshape(DC, 128, GROUP, 128).transpose(3, 2, 0, 1).reshape(
                128, GROUP * DC, 128
            )
        ).astype(BF16)

        in_maps.append(
            {
                "xt": xt,
                "wq": wq_host,
                "wk": wk_host,
                "wv": wv_host,
                "wo": wo_host,
                "bq": np.ascontiguousarray(bq_g.reshape(GROUP, 128).T),
                "bk": bk_g.reshape(128, 1).copy(),
                "bv": bv_g.reshape(128, 1).copy(),
                "tri": tri01,
            }
        )
    return in_maps


def kernel(
    hidden_states, Wq, bq, Wk, bk, Wv, bv, Wo, bo, _trace=False, _result_box=None
):
    from concourse.bass_utils import run_bass_kernel_spmd

    if "nc" not in _program_cache:
        _program_cache["nc"] = _build_program()
    nc = _program_cache["nc"]

    in_maps = _prep_inputs(hidden_states, Wq, bq, Wk, bk, Wv, bv, Wo, bo)
    res = run_bass_kernel_spmd(
        nc, in_maps, core_ids=list(range(NC_)), trace=_trace
    )
    if _result_box is not None:
        _result_box.append(res)

    acc = np.zeros((128, DC, NT), dtype=np.float32)
    for r in res.results:
        acc += np.asarray(r["out"], dtype=np.float32)
    # outT[Do*128+p, t] = acc[p, Do, t];  out[t, :] = outT[:, t] + bo
    outT = acc.transpose(1, 0, 2).reshape(D, NT)
    out = outT.T + np.asarray(bo, dtype=np.float32)[None, :]
    return np.ascontiguousarray(out.reshape(B, T, D), dtype=np.float32)
